# revision 8
# baseline (speedup 1.0000x reference)
"""Trainium2 Bass kernel for nn_DRNLayer (8-core batch-sharded, Chebyshev).

Math: out[i,j,l] = softmax_l( sum_k phi_ikl(w[j,k]) + B[j,l] ) where
  phi_ikl(w) = ln( sum_m exp(w*logD[l,m]) * P[i,k,m] )
is analytic in the scalar w.  Over the weight range [-wmax, wmax]
(wmax ~ 0.4) its degree-(S-1) Chebyshev interpolant is accurate to
~1e-3 at S=3 (higher S amplifies fp16 moment noise via larger
Lagrange cardinals), so

  logsum[i,j,l] ~= sum_{s,k} Lam_s[j,k] * phi_s[i,k,l]

with Lam_s[j,k] = Lagrange cardinal l_s(w[j,k]) computed on host.

Key latency facts this schedule is built around (measured):
  - every dma_start's completion semaphore fires ~2.3us after the
    engine instruction retires (HBM receipt latency), so consumers of
    input data cannot start before ~10.2us no matter what; inputs are
    therefore fused into as few DMAs as possible, all issued in the
    first microsecond, and the dead window is spent warming the PE
    clock gate (HAM) with filler matmuls so the real matmuls run at
    2.4 GHz instead of 1.2.
  - the softmax shift is a host-computed per-j constant (logits vary
    only ~+-3 around it), DMA'd in and fused into the ACT Exp via the
    per-partition bias operand: no max, no subtract on DVE.
  - outputs fan out over the gpsimd/sync/scalar queues so the last
    DMA instruction retires as early as possible (the ~2.3us receipt
    + ~7.6us engine-teardown epilogue after it are fixed costs).

Sharding: data-parallel over batch i (8 per core); parameters
replicated.  No collectives; host concatenates per-core outputs.
"""

import sys

sys.path.insert(0, "/opt/trn_rl_repo")

from contextlib import ExitStack

import numpy as np

import concourse.bacc as bacc
import concourse.bass as bass
import concourse.mybir as mybir
from concourse.bass_utils import run_bass_kernel_spmd
from concourse.tile import TileContext

F32 = mybir.dt.float32
F32R = mybir.dt.float32r
F16 = mybir.dt.float16
I32 = mybir.dt.int32
AF = mybir.ActivationFunctionType
ALU = mybir.AluOpType

N_CORES = 8
BATCH = 64
IB = BATCH // N_CORES  # 8 batch rows per core
IH = IB // 2  # half (4 i) granularity for Ln/step3/Exp
IQ = IB // 4  # quarter (2 i) granularity for the softmax tail
NJ = 128  # n_upper (all j on every core)
NK = 128  # n_lower
Q = 64  # q_upper == q_lower
S = 3  # Chebyshev nodes
SQ = S * Q
L0 = 32  # column the host shift is computed at
N_FILL = 11  # HAM-warming f16 filler matmuls during the input-DMA wait
ACT_TABLE_LN_EXP = 6  # act_info.json index of natural_log_exp_and_others

_NC = None
LAST_RESULTS = None


def _build():
    nc = bacc.Bacc()
    # IN2[p, 0:512]  = P[i, k, m] at [m + 64*(i%2), (i//2)*128 + k]
    # IN2[p, 512:704] = exp(ws[s]*logD[l, m]) at [m + 64*c, s*64 + l]
    I_d = nc.declare_dram_parameter("IN2", [128, 704], F16, isOutput=False)
    # LB[k, 0:384] = Lam[s, j, k] at [k, s*128 + j]
    # LB[l, 384:512] = B[j, l] for l < 64, zeros for l >= 64
    L_d = nc.declare_dram_parameter("LB", [128, 512], F32R, isOutput=False)
    C_d = nc.declare_dram_parameter("NEGC", [128, 8], F32, isOutput=False)
    o_d = nc.declare_dram_parameter("out", [NJ, IB, Q], F32, isOutput=True)

    with TileContext(nc) as tc, ExitStack() as ctx:
        consts = ctx.enter_context(tc.tile_pool(name="consts", bufs=1))
        phip = ctx.enter_context(tc.tile_pool(name="phi", bufs=1))
        spool = ctx.enter_context(tc.tile_pool(name="smax", bufs=1))

        # keep Exp+Ln resident in one ACT table for the whole kernel
        nc.scalar.add_instruction(
            mybir.InstLoadActFuncSet(
                name=nc.get_next_instruction_name(),
                ins=[],
                outs=[],
                act_func_set_id=ACT_TABLE_LN_EXP,
            )
        )

        # ---------------- input DMAs ----------------
        IN2 = consts.tile([128, 704], F16)
        nc.sync.dma_start(out=IN2, in_=I_d[:, :])
        LB = consts.tile([128, 512], F32R)
        nc.gpsimd.dma_start(out=LB, in_=L_d[:, :])
        negc = consts.tile([128, 8], F32)
        nc.scalar.dma_start(out=negc, in_=C_d[:, :])

        PT2 = IN2[:, 0:512].rearrange("p (a b) -> p a b", b=NK)
        DS2 = IN2[:, 512:704]
        LAMt = LB[:, 0:384].rearrange("p (s j) -> p s j", j=NJ)
        BT2 = LB[:, 384:512]

        # eye2[p, i, l] = (l == p) (zero rows for p >= 64): K=128 rhs of
        # the bias matmul so it runs on the full PE array
        it = consts.tile([128, Q], I32)
        nc.gpsimd.iota(it, pattern=[[1, Q]], base=0, channel_multiplier=-1)
        eye = consts.tile([128, Q], F32)
        nc.vector.tensor_scalar(eye, it, 0, None, ALU.is_equal)
        eye2 = consts.tile([128, IB, Q], F32R)
        nc.vector.tensor_copy(
            out=eye2, in_=eye.unsqueeze(1).broadcast_to([128, IB, Q])
        )

        PHI = phip.tile([NK, IB, SQ], F32R)

        # single PSUM pool for the whole kernel: a pool close would emit a
        # PE-queue DRAIN barrier stalling step3 behind every Ln
        psp = ctx.enter_context(tc.tile_pool(name="ps", bufs=1, space="PSUM"))
        ps1 = psp.tile([NK, IB, 512], F32)

        # ---------------- HAM warmers during the DMA-sem wait ----------
        # the input semaphores cannot fire before ~10.2us; keep the PE
        # busy meanwhile so its clock gate opens (2.4 GHz) for real work
        wz = consts.tile([128, 256], F32)
        nc.vector.memset(wz, 0.001)
        wzh = consts.tile([128, 256], F16)
        nc.vector.tensor_copy(out=wzh, in_=wz)
        for _ in range(N_FILL):
            nc.tensor.matmul(
                out=ps1[:, 7, 256 : 256 + SQ],
                lhsT=wzh[:, 0:128],
                rhs=wzh[:, 0:SQ],
                start=True,
                stop=True,
                skip_group_check=True,
            )

        # ---------------- step 1: moments ----------------
        for p in range(IB // 2):
            for c in range(2):  # row-tiled half-array matmuls, 2 i at once
                nc.tensor.matmul(
                    out=ps1[:, 2 * p + c, 0:SQ],
                    lhsT=PT2[64 * c : 64 * (c + 1), p, :],
                    rhs=DS2[64 * c : 64 * (c + 1), :],
                    start=True,
                    stop=True,
                    skip_group_check=True,
                )
            if p % 2 == 1:  # one Ln per i-half (4 PSUM banks)
                nc.scalar.activation(
                    out=PHI[:, 2 * p - 2 : 2 * p + 2, :],
                    in_=ps1[:, 2 * p - 2 : 2 * p + 2, 0:SQ],
                    func=AF.Ln,
                )

        # keep the PE busy through the Ln0 window so the HAM clock-gate
        # opens (~3.4us of sustained activity) before the step3 matmuls
        for nf in range(5):
            nc.tensor.matmul(
                out=ps1[:, 7, 256 : 256 + (SQ if nf < 4 else 64)],
                lhsT=wzh[:, 0:128],
                rhs=wzh[:, 0 : (SQ if nf < 4 else 64)],
                start=True,
                stop=True,
                skip_group_check=True,
            )

        # ---------------- step 3 + softmax ----------------
        for h in range(2):
            isl = slice(h * IH, (h + 1) * IH)
            out_h = ps1[:, 2 * h, 0 : IH * Q]
            lgh = out_h.rearrange("p (a b) -> p a b", b=Q)
            for s in range(S):
                nc.tensor.matmul(
                    out=out_h,
                    lhsT=LAMt[:, s, :],
                    rhs=PHI[:, isl, s * Q : (s + 1) * Q],
                    start=(s == 0),
                    stop=False,
                    skip_group_check=True,
                )
            nc.tensor.matmul(
                out=out_h,
                lhsT=BT2,
                rhs=eye2[:, isl, :],
                start=False,
                stop=True,
                skip_group_check=True,
            )
            # exp(logits - c) with the host shift fused in as ACT bias;
            # softmax is exactly shift-invariant and |logits-c| < ~3
            em = spool.tile([NJ, IH, Q], F32, tag=f"em{h}")
            nc.scalar.activation(
                out=em, in_=lgh, func=AF.Exp, bias=negc[:, 0:1]
            )
            if h == 0:
                # whole-half tail on gpsimd + its DMA queue
                sm = spool.tile([NJ, IH], F32, tag="sm0")
                nc.vector.tensor_reduce(
                    sm, em, axis=mybir.AxisListType.X, op=ALU.add
                )
                rec = spool.tile([NJ, IH], F32, tag="rec0")
                nc.vector.reciprocal(rec, sm)
                oute = spool.tile([NJ, IH, Q], F32, tag="oute0")
                nc.gpsimd.tensor_tensor(
                    out=oute,
                    in0=em,
                    in1=rec.unsqueeze(2).broadcast_to([NJ, IH, Q]),
                    op=ALU.mult,
                )
                nc.gpsimd.dma_start(out=o_d[:, 0:IH, :], in_=oute)
            else:
                # per-2i tail on DVE, DMAs on sync + scalar queues
                for qq in range(2):
                    i0 = IH + IQ * qq
                    emq = em[:, IQ * qq : IQ * qq + IQ, :]
                    sm = spool.tile([NJ, IQ], F32, tag=f"sm{qq + 1}")
                    nc.vector.tensor_reduce(
                        sm, emq, axis=mybir.AxisListType.X, op=ALU.add
                    )
                    rec = spool.tile([NJ, IQ], F32, tag=f"rec{qq + 1}")
                    nc.vector.reciprocal(rec, sm)
                    oute = spool.tile([NJ, IQ, Q], F32, tag=f"oute{qq + 1}")
                    nc.vector.tensor_tensor(
                        out=oute,
                        in0=emq,
                        in1=rec.unsqueeze(2).broadcast_to([NJ, IQ, Q]),
                        op=ALU.mult,
                    )
                    qeng = nc.sync if qq == 0 else nc.scalar
                    qeng.dma_start(out=o_d[:, i0 : i0 + IQ, :], in_=oute)

    nc.compile()
    return nc


def kernel(P, weight, bias_abs, bias_q, lambda_abs, lambda_q):
    global _NC, LAST_RESULTS
    P = np.asarray(P, dtype=np.float32)
    weight = np.asarray(weight, dtype=np.float32)
    bias_abs = np.asarray(bias_abs, dtype=np.float32)
    bias_q = np.asarray(bias_q, dtype=np.float32)
    lambda_abs = np.asarray(lambda_abs, dtype=np.float32)
    lambda_q = np.asarray(lambda_q, dtype=np.float32)

    if _NC is None:
        _NC = _build()

    qv = np.arange(Q, dtype=np.float32) / Q
    logD = -(qv[None, :] - qv[:, None]) ** 2  # [l, m]

    wmax = float(np.abs(weight).max())
    if wmax == 0.0:
        wmax = 1e-6
    ws = np.cos((2 * np.arange(S) + 1) / (2 * S) * np.pi) * wmax  # [S]

    Ds = np.exp(ws[:, None, None] * logD[None, :, :])  # [S, l, m]
    DS = Ds.transpose(2, 0, 1).reshape(Q, SQ).astype(np.float16)  # [m, (s,l)]
    DS2 = np.concatenate([DS, DS], axis=0)  # [128, SQ]

    Lam = np.ones((S,) + weight.shape, dtype=np.float64)  # [S, j, k]
    for s in range(S):
        for r in range(S):
            if r != s:
                Lam[s] *= (weight - ws[r]) / (ws[s] - ws[r])
    LAM = Lam.transpose(2, 0, 1).astype(np.float32).reshape(NK, S * NJ)  # [k,(s,j)]

    sv = qv[None, :]  # [1, 64]
    B = (-bias_q * (sv - lambda_q) ** 2 - bias_abs * np.abs(sv - lambda_abs)).astype(
        np.float32
    )  # [j, l]
    BT2 = np.zeros((128, NJ), dtype=np.float32)
    BT2[0:Q] = B.T
    LB = np.ascontiguousarray(np.concatenate([LAM, BT2], axis=1))  # [128, 512]

    # host softmax shift: c[j] ~ logits[i,j,L0] to within ~+-3
    c = (
        np.log(0.5 * np.exp(weight[:, :, None] * logD[None, None, L0, :]).sum(-1)).sum(
            1
        )
        + B[:, L0]
    ).astype(np.float32)
    NEGC = np.zeros((128, 8), dtype=np.float32)
    NEGC[:, 0] = -c

    # PT2[m + 64*(i%2), i//2, k] = P[i, k, m] per core slice
    PTfull = P.transpose(2, 0, 1)  # [m, i, k]

    in_maps = []
    for cc in range(N_CORES):
        sl = PTfull[:, cc * IB : (cc + 1) * IB, :]  # [64, 8, 128]
        PT2 = np.empty((128, IB // 2, NK), dtype=np.float16)
        PT2[0:64] = sl[:, 0::2, :]
        PT2[64:128] = sl[:, 1::2, :]
        IN2 = np.concatenate([PT2.reshape(128, 512), DS2], axis=1)  # [128, 704]
        in_maps.append(
            {
                "IN2": np.ascontiguousarray(IN2),
                "LB": LB,
                "NEGC": NEGC,
            }
        )

    LAST_RESULTS = run_bass_kernel_spmd(_NC, in_maps, list(range(N_CORES)))
    out = np.empty((BATCH, NJ, Q), dtype=np.float32)
    for cc in range(N_CORES):
        o = LAST_RESULTS.results[cc]["out"]  # [j, i, l]
        out[cc * IB : (cc + 1) * IB] = o.transpose(1, 0, 2)
    return out


# revision 10
# speedup vs baseline: 1.0057x; 1.0057x over previous
"""Trainium2 Bass kernel for nn_DRNLayer (8-core batch-sharded, Chebyshev).

Math: out[i,j,l] = softmax_l( sum_k phi_ikl(w[j,k]) + B[j,l] ) where
  phi_ikl(w) = ln( sum_m exp(w*logD[l,m]) * P[i,k,m] )
is analytic in the scalar w.  Over the weight range [-wmax, wmax]
(wmax ~ 0.4) its degree-(S-1) Chebyshev interpolant is accurate to
~1e-3 at S=3 (higher S amplifies fp16 moment noise via larger
Lagrange cardinals), so

  logsum[i,j,l] ~= sum_{s,k} Lam_s[j,k] * phi_s[i,k,l]

with Lam_s[j,k] = Lagrange cardinal l_s(w[j,k]) computed on host.

Key latency facts this schedule is built around (measured):
  - every dma_start's completion semaphore fires ~2.3us after the
    engine instruction retires (HBM receipt latency), so consumers of
    input data cannot start before ~10.2us no matter what; inputs are
    therefore fused into as few DMAs as possible, all issued in the
    first microsecond, and the dead window is spent warming the PE
    clock gate (HAM) with filler matmuls so the real matmuls run at
    2.4 GHz instead of 1.2.
  - the softmax shift is a host-computed per-j constant (logits vary
    only ~+-3 around it), DMA'd in and fused into the ACT Exp via the
    per-partition bias operand: no max, no subtract on DVE.
  - outputs fan out over the gpsimd/sync/scalar queues so the last
    DMA instruction retires as early as possible (the ~2.3us receipt
    + ~7.6us engine-teardown epilogue after it are fixed costs).

Sharding: data-parallel over batch i (8 per core); parameters
replicated.  No collectives; host concatenates per-core outputs.
"""

import sys

sys.path.insert(0, "/opt/trn_rl_repo")

from contextlib import ExitStack

import numpy as np

import concourse.bacc as bacc
import concourse.bass as bass
import concourse.mybir as mybir
from concourse.bass_utils import run_bass_kernel_spmd
from concourse.tile import TileContext

F32 = mybir.dt.float32
F32R = mybir.dt.float32r
F16 = mybir.dt.float16
I32 = mybir.dt.int32
AF = mybir.ActivationFunctionType
ALU = mybir.AluOpType

N_CORES = 8
BATCH = 64
IB = BATCH // N_CORES  # 8 batch rows per core
IH = IB // 2  # half (4 i) granularity for Ln/step3/Exp
IQ = IB // 4  # quarter (2 i) granularity for the softmax tail
NJ = 128  # n_upper (all j on every core)
NK = 128  # n_lower
Q = 64  # q_upper == q_lower
S = 3  # Chebyshev nodes
SQ = S * Q
L0 = 32  # column the host shift is computed at
N_FILL = 11  # HAM-warming f16 filler matmuls during the input-DMA wait
ACT_TABLE_LN_EXP = 6  # act_info.json index of natural_log_exp_and_others

_NC = None
LAST_RESULTS = None


def _build():
    nc = bacc.Bacc()
    # IN2[p, 0:512]  = P[i, k, m] at [m + 64*(i%2), (i//2)*128 + k]
    # IN2[p, 512:704] = exp(ws[s]*logD[l, m]) at [m + 64*c, s*64 + l]
    I_d = nc.declare_dram_parameter("IN2", [128, 704], F16, isOutput=False)
    # LB[k, 0:384] = Lam[s, j, k] at [k, s*128 + j]
    # LB[l, 384:512] = B[j, l] for l < 64, zeros for l >= 64
    L_d = nc.declare_dram_parameter("LB", [128, 512], F32R, isOutput=False)
    C_d = nc.declare_dram_parameter("NEGC", [128, 8], F32, isOutput=False)
    o_d = nc.declare_dram_parameter("out", [NJ, IB, Q], F32, isOutput=True)

    with TileContext(nc) as tc, ExitStack() as ctx:
        consts = ctx.enter_context(tc.tile_pool(name="consts", bufs=1))
        phip = ctx.enter_context(tc.tile_pool(name="phi", bufs=1))
        spool = ctx.enter_context(tc.tile_pool(name="smax", bufs=1))

        # keep Exp+Ln resident in one ACT table for the whole kernel
        nc.scalar.add_instruction(
            mybir.InstLoadActFuncSet(
                name=nc.get_next_instruction_name(),
                ins=[],
                outs=[],
                act_func_set_id=ACT_TABLE_LN_EXP,
            )
        )

        # ---------------- input DMAs ----------------
        IN2 = consts.tile([128, 704], F16)
        nc.sync.dma_start(out=IN2, in_=I_d[:, :])
        LB = consts.tile([128, 512], F32R)
        nc.gpsimd.dma_start(out=LB, in_=L_d[:, :])
        negc = consts.tile([128, 8], F32)
        nc.scalar.dma_start(out=negc, in_=C_d[:, :])

        PT2 = IN2[:, 0:512].rearrange("p (a b) -> p a b", b=NK)
        DS2 = IN2[:, 512:704]
        LAMt = LB[:, 0:384].rearrange("p (s j) -> p s j", j=NJ)
        BT2 = LB[:, 384:512]

        # eye2[p, i, l] = (l == p) (zero rows for p >= 64): K=128 rhs of
        # the bias matmul so it runs on the full PE array
        it = consts.tile([128, Q], I32)
        nc.gpsimd.iota(it, pattern=[[1, Q]], base=0, channel_multiplier=-1)
        eye = consts.tile([128, Q], F32)
        nc.vector.tensor_scalar(eye, it, 0, None, ALU.is_equal)
        eye2 = consts.tile([128, IB, Q], F32R)
        nc.vector.tensor_copy(
            out=eye2, in_=eye.unsqueeze(1).broadcast_to([128, IB, Q])
        )

        PHI = phip.tile([NK, IB, SQ], F32R)

        # single PSUM pool for the whole kernel: a pool close would emit a
        # PE-queue DRAIN barrier stalling step3 behind every Ln
        psp = ctx.enter_context(tc.tile_pool(name="ps", bufs=1, space="PSUM"))
        ps1 = psp.tile([NK, IB, 512], F32)

        # ---------------- HAM warmers during the DMA-sem wait ----------
        # the input semaphores cannot fire before ~10.2us; keep the PE
        # busy meanwhile so its clock gate opens (2.4 GHz) for real work
        wz = consts.tile([128, 256], F32)
        nc.vector.memset(wz, 0.001)
        wzh = consts.tile([128, 256], F16)
        nc.vector.tensor_copy(out=wzh, in_=wz)
        for _ in range(N_FILL):
            nc.tensor.matmul(
                out=ps1[:, 7, 256 : 256 + SQ],
                lhsT=wzh[:, 0:128],
                rhs=wzh[:, 0:SQ],
                start=True,
                stop=True,
                skip_group_check=True,
            )

        # ---------------- step 1: moments ----------------
        for p in range(IB // 2):
            for c in range(2):  # row-tiled half-array matmuls, 2 i at once
                nc.tensor.matmul(
                    out=ps1[:, 2 * p + c, 0:SQ],
                    lhsT=PT2[64 * c : 64 * (c + 1), p, :],
                    rhs=DS2[64 * c : 64 * (c + 1), :],
                    start=True,
                    stop=True,
                    skip_group_check=True,
                )
            if p % 2 == 1:  # one Ln per i-half (4 PSUM banks)
                nc.scalar.activation(
                    out=PHI[:, 2 * p - 2 : 2 * p + 2, :],
                    in_=ps1[:, 2 * p - 2 : 2 * p + 2, 0:SQ],
                    func=AF.Ln,
                )



        # ---------------- step 3 + softmax ----------------
        for h in range(2):
            isl = slice(h * IH, (h + 1) * IH)
            out_h = ps1[:, 2 * h, 0 : IH * Q]
            lgh = out_h.rearrange("p (a b) -> p a b", b=Q)
            for s in range(S):
                nc.tensor.matmul(
                    out=out_h,
                    lhsT=LAMt[:, s, :],
                    rhs=PHI[:, isl, s * Q : (s + 1) * Q],
                    start=(s == 0),
                    stop=False,
                    skip_group_check=True,
                )
            nc.tensor.matmul(
                out=out_h,
                lhsT=BT2,
                rhs=eye2[:, isl, :],
                start=False,
                stop=True,
                skip_group_check=True,
            )
            # exp(logits - c) with the host shift fused in as ACT bias;
            # softmax is exactly shift-invariant and |logits-c| < ~3
            if h == 0:
                # whole-half Exp, tail on gpsimd + its DMA queue
                em = spool.tile([NJ, IH, Q], F32, tag="em0")
                nc.scalar.activation(
                    out=em, in_=lgh, func=AF.Exp, bias=negc[:, 0:1]
                )
                sm = spool.tile([NJ, IH], F32, tag="sm0")
                nc.vector.tensor_reduce(
                    sm, em, axis=mybir.AxisListType.X, op=ALU.add
                )
                rec = spool.tile([NJ, IH], F32, tag="rec0")
                nc.vector.reciprocal(rec, sm)
                oute = spool.tile([NJ, IH, Q], F32, tag="oute0")
                nc.gpsimd.tensor_tensor(
                    out=oute,
                    in0=em,
                    in1=rec.unsqueeze(2).broadcast_to([NJ, IH, Q]),
                    op=ALU.mult,
                )
                nc.gpsimd.dma_start(out=o_d[:, 0:IH, :], in_=oute)
            else:
                # per-2i Exp + tail on DVE, DMAs on sync + scalar queues
                for qq in range(2):
                    i0 = IH + IQ * qq
                    em = spool.tile([NJ, IQ, Q], F32, tag=f"em{qq + 1}")
                    nc.scalar.activation(
                        out=em,
                        in_=lgh[:, IQ * qq : IQ * qq + IQ, :],
                        func=AF.Exp,
                        bias=negc[:, 0:1],
                    )
                    sm = spool.tile([NJ, IQ], F32, tag=f"sm{qq + 1}")
                    nc.vector.tensor_reduce(
                        sm, em, axis=mybir.AxisListType.X, op=ALU.add
                    )
                    rec = spool.tile([NJ, IQ], F32, tag=f"rec{qq + 1}")
                    nc.vector.reciprocal(rec, sm)
                    oute = spool.tile([NJ, IQ, Q], F32, tag=f"oute{qq + 1}")
                    nc.vector.tensor_tensor(
                        out=oute,
                        in0=em,
                        in1=rec.unsqueeze(2).broadcast_to([NJ, IQ, Q]),
                        op=ALU.mult,
                    )
                    qeng = nc.sync if qq == 0 else nc.scalar
                    qeng.dma_start(out=o_d[:, i0 : i0 + IQ, :], in_=oute)

    nc.compile()
    return nc


def kernel(P, weight, bias_abs, bias_q, lambda_abs, lambda_q):
    global _NC, LAST_RESULTS
    P = np.asarray(P, dtype=np.float32)
    weight = np.asarray(weight, dtype=np.float32)
    bias_abs = np.asarray(bias_abs, dtype=np.float32)
    bias_q = np.asarray(bias_q, dtype=np.float32)
    lambda_abs = np.asarray(lambda_abs, dtype=np.float32)
    lambda_q = np.asarray(lambda_q, dtype=np.float32)

    if _NC is None:
        _NC = _build()

    qv = np.arange(Q, dtype=np.float32) / Q
    logD = -(qv[None, :] - qv[:, None]) ** 2  # [l, m]

    wmax = float(np.abs(weight).max())
    if wmax == 0.0:
        wmax = 1e-6
    ws = np.cos((2 * np.arange(S) + 1) / (2 * S) * np.pi) * wmax  # [S]

    Ds = np.exp(ws[:, None, None] * logD[None, :, :])  # [S, l, m]
    DS = Ds.transpose(2, 0, 1).reshape(Q, SQ).astype(np.float16)  # [m, (s,l)]
    DS2 = np.concatenate([DS, DS], axis=0)  # [128, SQ]

    Lam = np.ones((S,) + weight.shape, dtype=np.float64)  # [S, j, k]
    for s in range(S):
        for r in range(S):
            if r != s:
                Lam[s] *= (weight - ws[r]) / (ws[s] - ws[r])
    LAM = Lam.transpose(2, 0, 1).astype(np.float32).reshape(NK, S * NJ)  # [k,(s,j)]

    sv = qv[None, :]  # [1, 64]
    B = (-bias_q * (sv - lambda_q) ** 2 - bias_abs * np.abs(sv - lambda_abs)).astype(
        np.float32
    )  # [j, l]
    BT2 = np.zeros((128, NJ), dtype=np.float32)
    BT2[0:Q] = B.T
    LB = np.ascontiguousarray(np.concatenate([LAM, BT2], axis=1))  # [128, 512]

    # host softmax shift: c[j] ~ logits[i,j,L0] to within ~+-3
    c = (
        np.log(0.5 * np.exp(weight[:, :, None] * logD[None, None, L0, :]).sum(-1)).sum(
            1
        )
        + B[:, L0]
    ).astype(np.float32)
    NEGC = np.zeros((128, 8), dtype=np.float32)
    NEGC[:, 0] = -c

    # PT2[m + 64*(i%2), i//2, k] = P[i, k, m] per core slice
    PTfull = P.transpose(2, 0, 1)  # [m, i, k]

    in_maps = []
    for cc in range(N_CORES):
        sl = PTfull[:, cc * IB : (cc + 1) * IB, :]  # [64, 8, 128]
        PT2 = np.empty((128, IB // 2, NK), dtype=np.float16)
        PT2[0:64] = sl[:, 0::2, :]
        PT2[64:128] = sl[:, 1::2, :]
        IN2 = np.concatenate([PT2.reshape(128, 512), DS2], axis=1)  # [128, 704]
        in_maps.append(
            {
                "IN2": np.ascontiguousarray(IN2),
                "LB": LB,
                "NEGC": NEGC,
            }
        )

    LAST_RESULTS = run_bass_kernel_spmd(_NC, in_maps, list(range(N_CORES)))
    out = np.empty((BATCH, NJ, Q), dtype=np.float32)
    for cc in range(N_CORES):
        o = LAST_RESULTS.results[cc]["out"]  # [j, i, l]
        out[cc * IB : (cc + 1) * IB] = o.transpose(1, 0, 2)
    return out


# revision 11
# speedup vs baseline: 1.0221x; 1.0164x over previous
"""Trainium2 Bass kernel for nn_DRNLayer (8-core batch-sharded, Chebyshev).

Math: out[i,j,l] = softmax_l( sum_k phi_ikl(w[j,k]) + B[j,l] ) where
  phi_ikl(w) = ln( sum_m exp(w*logD[l,m]) * P[i,k,m] )
is analytic in the scalar w.  Over the weight range [-wmax, wmax]
(wmax ~ 0.4) its degree-(S-1) Chebyshev interpolant is accurate to
~1e-3 at S=3 (higher S amplifies fp16 moment noise via larger
Lagrange cardinals), so

  logsum[i,j,l] ~= sum_{s,k} Lam_s[j,k] * phi_s[i,k,l]

with Lam_s[j,k] = Lagrange cardinal l_s(w[j,k]) computed on host.

Key latency facts this schedule is built around (measured):
  - every dma_start's completion semaphore fires ~2.3us after the
    engine instruction retires (HBM receipt latency), so consumers of
    input data cannot start before ~10.2us no matter what; inputs are
    therefore fused into as few DMAs as possible, all issued in the
    first microsecond, and the dead window is spent warming the PE
    clock gate (HAM) with filler matmuls so the real matmuls run at
    2.4 GHz instead of 1.2.
  - the softmax shift is a host-computed per-j constant (logits vary
    only ~+-3 around it), DMA'd in and fused into the ACT Exp via the
    per-partition bias operand: no max, no subtract on DVE.
  - outputs fan out over the gpsimd/sync/scalar queues so the last
    DMA instruction retires as early as possible (the ~2.3us receipt
    + ~7.6us engine-teardown epilogue after it are fixed costs).

Sharding: data-parallel over batch i (8 per core); parameters
replicated.  No collectives; host concatenates per-core outputs.
"""

import sys

sys.path.insert(0, "/opt/trn_rl_repo")

from contextlib import ExitStack

import numpy as np

import concourse.bacc as bacc
import concourse.bass as bass
import concourse.mybir as mybir
from concourse.bass_utils import run_bass_kernel_spmd
from concourse.tile import TileContext

F32 = mybir.dt.float32
F32R = mybir.dt.float32r
F16 = mybir.dt.float16
I32 = mybir.dt.int32
AF = mybir.ActivationFunctionType
ALU = mybir.AluOpType

N_CORES = 8
BATCH = 64
IB = BATCH // N_CORES  # 8 batch rows per core
IH = IB // 2  # half (4 i) granularity for Ln/step3/Exp
IQ = IB // 4  # quarter (2 i) granularity for the softmax tail
NJ = 128  # n_upper (all j on every core)
NK = 128  # n_lower
Q = 64  # q_upper == q_lower
S = 3  # Chebyshev nodes
SQ = S * Q
L0 = 32  # column the host shift is computed at
N_FILL = 11  # HAM-warming f16 filler matmuls during the input-DMA wait
ACT_TABLE_LN_EXP = 6  # act_info.json index of natural_log_exp_and_others

_NC = None
LAST_RESULTS = None


def _build():
    nc = bacc.Bacc()
    # IN2[p, 0:512]  = P[i, k, m] at [m + 64*(i%2), (i//2)*128 + k]
    # IN2[p, 512:704] = exp(ws[s]*logD[l, m]) at [m + 64*c, s*64 + l]
    I_d = nc.declare_dram_parameter("IN2", [128, 704], F16, isOutput=False)
    # LB[k, 0:384] = Lam[s, j, k] at [k, s*128 + j]
    # LB[l, 384:512] = B[j, l] for l < 64, zeros for l >= 64
    L_d = nc.declare_dram_parameter("LB", [128, 512], F32R, isOutput=False)
    C_d = nc.declare_dram_parameter("NEGC", [128, 8], F32, isOutput=False)
    o_d = nc.declare_dram_parameter("out", [NJ, IB, Q], F32, isOutput=True)

    with TileContext(nc) as tc, ExitStack() as ctx:
        consts = ctx.enter_context(tc.tile_pool(name="consts", bufs=1))
        phip = ctx.enter_context(tc.tile_pool(name="phi", bufs=1))
        spool = ctx.enter_context(tc.tile_pool(name="smax", bufs=1))

        # keep Exp+Ln resident in one ACT table for the whole kernel
        nc.scalar.add_instruction(
            mybir.InstLoadActFuncSet(
                name=nc.get_next_instruction_name(),
                ins=[],
                outs=[],
                act_func_set_id=ACT_TABLE_LN_EXP,
            )
        )

        # ---------------- input DMAs ----------------
        IN2 = consts.tile([128, 704], F16)
        nc.sync.dma_start(out=IN2, in_=I_d[:, :])
        LB = consts.tile([128, 512], F32R)
        nc.gpsimd.dma_start(out=LB, in_=L_d[:, :])
        negc = consts.tile([128, 8], F32)
        nc.scalar.dma_start(out=negc, in_=C_d[:, :])

        PT2 = IN2[:, 0:512].rearrange("p (a b) -> p a b", b=NK)
        DS2 = IN2[:, 512:704]
        LAMt = LB[:, 0:384].rearrange("p (s j) -> p s j", j=NJ)
        BT2 = LB[:, 384:512]

        # eye2[p, i, l] = (l == p) (zero rows for p >= 64): K=128 rhs of
        # the bias matmul so it runs on the full PE array
        it = consts.tile([128, Q], I32)
        nc.gpsimd.iota(it, pattern=[[1, Q]], base=0, channel_multiplier=-1)
        eye = consts.tile([128, Q], F32)
        nc.vector.tensor_scalar(eye, it, 0, None, ALU.is_equal)
        eye2 = consts.tile([128, IB, Q], F32R)
        nc.vector.tensor_copy(
            out=eye2, in_=eye.unsqueeze(1).broadcast_to([128, IB, Q])
        )

        PHI = phip.tile([NK, IB, SQ], F32R)

        # single PSUM pool for the whole kernel: a pool close would emit a
        # PE-queue DRAIN barrier stalling step3 behind every Ln
        psp = ctx.enter_context(tc.tile_pool(name="ps", bufs=1, space="PSUM"))
        ps1 = psp.tile([NK, IB, 512], F32)

        # ---------------- HAM warmers during the DMA-sem wait ----------
        # the input semaphores cannot fire before ~10.2us; keep the PE
        # busy meanwhile so its clock gate opens (2.4 GHz) for real work
        wz = consts.tile([128, 256], F32)
        nc.vector.memset(wz, 0.001)
        wzh = consts.tile([128, 256], F16)
        nc.vector.tensor_copy(out=wzh, in_=wz)
        for _ in range(N_FILL):
            nc.tensor.matmul(
                out=ps1[:, 7, 256 : 256 + SQ],
                lhsT=wzh[:, 0:128],
                rhs=wzh[:, 0:SQ],
                start=True,
                stop=True,
                skip_group_check=True,
            )

        # ---------------- step 1: moments ----------------
        for p in range(IB // 2):
            for c in range(2):  # row-tiled half-array matmuls, 2 i at once
                nc.tensor.matmul(
                    out=ps1[:, 2 * p + c, 0:SQ],
                    lhsT=PT2[64 * c : 64 * (c + 1), p, :],
                    rhs=DS2[64 * c : 64 * (c + 1), :],
                    start=True,
                    stop=True,
                    skip_group_check=True,
                )
            if p % 2 == 1:  # one Ln per i-half (4 PSUM banks)
                nc.scalar.activation(
                    out=PHI[:, 2 * p - 2 : 2 * p + 2, :],
                    in_=ps1[:, 2 * p - 2 : 2 * p + 2, 0:SQ],
                    func=AF.Ln,
                )



        # keep the PE busy through the Ln0 window so the HAM clock-gate
        # opens (~3.4us of sustained activity) before the step3 matmuls;
        # rhs comes from IN2 so these are ready exactly when step1 is and
        # the scheduler slots them after it, not before
        for nf in range(5):
            w = SQ if nf < 4 else 64
            nc.tensor.matmul(
                out=ps1[:, 7, 256 : 256 + w],
                lhsT=wzh[:, 0:128],
                rhs=IN2[:, 512 : 512 + w],
                start=True,
                stop=True,
                skip_group_check=True,
            )

        # ---------------- step 3 + softmax ----------------
        for h in range(2):
            isl = slice(h * IH, (h + 1) * IH)
            out_h = ps1[:, 2 * h, 0 : IH * Q]
            lgh = out_h.rearrange("p (a b) -> p a b", b=Q)
            for s in range(S):
                nc.tensor.matmul(
                    out=out_h,
                    lhsT=LAMt[:, s, :],
                    rhs=PHI[:, isl, s * Q : (s + 1) * Q],
                    start=(s == 0),
                    stop=False,
                    skip_group_check=True,
                )
            nc.tensor.matmul(
                out=out_h,
                lhsT=BT2,
                rhs=eye2[:, isl, :],
                start=False,
                stop=True,
                skip_group_check=True,
            )
            # exp(logits - c) with the host shift fused in as ACT bias;
            # softmax is exactly shift-invariant and |logits-c| < ~3
            if h == 0:
                # whole-half Exp, tail on gpsimd + its DMA queue
                em = spool.tile([NJ, IH, Q], F32, tag="em0")
                nc.scalar.activation(
                    out=em, in_=lgh, func=AF.Exp, bias=negc[:, 0:1]
                )
                sm = spool.tile([NJ, IH], F32, tag="sm0")
                nc.vector.tensor_reduce(
                    sm, em, axis=mybir.AxisListType.X, op=ALU.add
                )
                rec = spool.tile([NJ, IH], F32, tag="rec0")
                nc.vector.reciprocal(rec, sm)
                oute = spool.tile([NJ, IH, Q], F32, tag="oute0")
                nc.gpsimd.tensor_tensor(
                    out=oute,
                    in0=em,
                    in1=rec.unsqueeze(2).broadcast_to([NJ, IH, Q]),
                    op=ALU.mult,
                )
                nc.gpsimd.dma_start(out=o_d[:, 0:IH, :], in_=oute)
            else:
                # per-2i Exp + tail on DVE, DMAs on sync + scalar queues
                for qq in range(2):
                    i0 = IH + IQ * qq
                    em = spool.tile([NJ, IQ, Q], F32, tag=f"em{qq + 1}")
                    nc.scalar.activation(
                        out=em,
                        in_=lgh[:, IQ * qq : IQ * qq + IQ, :],
                        func=AF.Exp,
                        bias=negc[:, 0:1],
                    )
                    sm = spool.tile([NJ, IQ], F32, tag=f"sm{qq + 1}")
                    nc.vector.tensor_reduce(
                        sm, em, axis=mybir.AxisListType.X, op=ALU.add
                    )
                    rec = spool.tile([NJ, IQ], F32, tag=f"rec{qq + 1}")
                    nc.vector.reciprocal(rec, sm)
                    oute = spool.tile([NJ, IQ, Q], F32, tag=f"oute{qq + 1}")
                    nc.vector.tensor_tensor(
                        out=oute,
                        in0=em,
                        in1=rec.unsqueeze(2).broadcast_to([NJ, IQ, Q]),
                        op=ALU.mult,
                    )
                    qeng = nc.sync if qq == 0 else nc.scalar
                    qeng.dma_start(out=o_d[:, i0 : i0 + IQ, :], in_=oute)

    nc.compile()
    return nc


def kernel(P, weight, bias_abs, bias_q, lambda_abs, lambda_q):
    global _NC, LAST_RESULTS
    P = np.asarray(P, dtype=np.float32)
    weight = np.asarray(weight, dtype=np.float32)
    bias_abs = np.asarray(bias_abs, dtype=np.float32)
    bias_q = np.asarray(bias_q, dtype=np.float32)
    lambda_abs = np.asarray(lambda_abs, dtype=np.float32)
    lambda_q = np.asarray(lambda_q, dtype=np.float32)

    if _NC is None:
        _NC = _build()

    qv = np.arange(Q, dtype=np.float32) / Q
    logD = -(qv[None, :] - qv[:, None]) ** 2  # [l, m]

    wmax = float(np.abs(weight).max())
    if wmax == 0.0:
        wmax = 1e-6
    ws = np.cos((2 * np.arange(S) + 1) / (2 * S) * np.pi) * wmax  # [S]

    Ds = np.exp(ws[:, None, None] * logD[None, :, :])  # [S, l, m]
    DS = Ds.transpose(2, 0, 1).reshape(Q, SQ).astype(np.float16)  # [m, (s,l)]
    DS2 = np.concatenate([DS, DS], axis=0)  # [128, SQ]

    Lam = np.ones((S,) + weight.shape, dtype=np.float64)  # [S, j, k]
    for s in range(S):
        for r in range(S):
            if r != s:
                Lam[s] *= (weight - ws[r]) / (ws[s] - ws[r])
    LAM = Lam.transpose(2, 0, 1).astype(np.float32).reshape(NK, S * NJ)  # [k,(s,j)]

    sv = qv[None, :]  # [1, 64]
    B = (-bias_q * (sv - lambda_q) ** 2 - bias_abs * np.abs(sv - lambda_abs)).astype(
        np.float32
    )  # [j, l]
    BT2 = np.zeros((128, NJ), dtype=np.float32)
    BT2[0:Q] = B.T
    LB = np.ascontiguousarray(np.concatenate([LAM, BT2], axis=1))  # [128, 512]

    # host softmax shift: c[j] ~ logits[i,j,L0] to within ~+-3
    c = (
        np.log(0.5 * np.exp(weight[:, :, None] * logD[None, None, L0, :]).sum(-1)).sum(
            1
        )
        + B[:, L0]
    ).astype(np.float32)
    NEGC = np.zeros((128, 8), dtype=np.float32)
    NEGC[:, 0] = -c

    # PT2[m + 64*(i%2), i//2, k] = P[i, k, m] per core slice
    PTfull = P.transpose(2, 0, 1)  # [m, i, k]

    in_maps = []
    for cc in range(N_CORES):
        sl = PTfull[:, cc * IB : (cc + 1) * IB, :]  # [64, 8, 128]
        PT2 = np.empty((128, IB // 2, NK), dtype=np.float16)
        PT2[0:64] = sl[:, 0::2, :]
        PT2[64:128] = sl[:, 1::2, :]
        IN2 = np.concatenate([PT2.reshape(128, 512), DS2], axis=1)  # [128, 704]
        in_maps.append(
            {
                "IN2": np.ascontiguousarray(IN2),
                "LB": LB,
                "NEGC": NEGC,
            }
        )

    LAST_RESULTS = run_bass_kernel_spmd(_NC, in_maps, list(range(N_CORES)))
    out = np.empty((BATCH, NJ, Q), dtype=np.float32)
    for cc in range(N_CORES):
        o = LAST_RESULTS.results[cc]["out"]  # [j, i, l]
        out[cc * IB : (cc + 1) * IB] = o.transpose(1, 0, 2)
    return out
